# revision 6
# baseline (speedup 1.0000x reference)
"""Trainium2 Bass kernel for nn_EstraNet_1443109012284.

Mathematical reduction: the reference's FAVOR+/trig branch (phi_q, aux_q/k,
fr_q/k, aux_A, A) does not feed the output.  The output is exactly

    out[b,n,d] = sum_{h,c} W_o[h,c,d] * norma[h] * sum_{d'} W_v[d',h,c] * x[b,n,d']
               = (x @ M)[b,n,d],   M[d',d] = sum_{h,c} W_v[d',h,c] norma[h] W_o[h,c,d]

with norma[h] = || sum_d s_p[h] W_p[d,h,:] beta_p[d] ||_2.

M is a tiny [512,512] matrix folded on the host; the device does the single
big GEMM  y[32768,512] = x[32768,512] @ M[512,512]  data-parallel over rows:
each of the 8 cores handles 4096 rows.

Device layout per core: x is fed pre-transposed (k-major) so the contraction
dim lands on SBUF partitions without any on-device transpose.  lhsT = xT tile
[128k x 128n] (stationary), rhs = M chunk [128k x 512d] (moving), accumulate
4 k-chunks in PSUM, copy PSUM->SBUF, DMA out in 1 MB blocks.
"""

import sys

sys.path.insert(0, "/opt/trn_rl_repo")

import numpy as np

import concourse.bass as bass
import concourse.tile as tile
from concourse import bacc, mybir
from concourse.bass_utils import run_bass_kernel_spmd

N_CORES = 8
ROWS = 32768          # B*N = 8*4096
RPC = ROWS // N_CORES  # rows per core = 4096
D = 512
KC = 4                # contraction chunks of 128
NT = RPC // 128       # 32 n-tiles of 128 rows per core
OUT_GROUP = 4         # n-tiles per output DMA (1 MB per dma_start)

# device compute dtype: "fp32" (exact, 4 cyc/row), "bf16" (1 cyc/row),
# "f32r" (fp32 data, reduced-precision fast path)
COMPUTE_DTYPE = "fp32"

_DT = {
    "fp32": (mybir.dt.float32, np.float32),
    "f32r": (mybir.dt.float32r, np.float32),
    "bf16": (mybir.dt.bfloat16, None),  # numpy dtype resolved via ml_dtypes
}


def _np_dtype(token):
    if token == "bf16":
        import ml_dtypes

        return ml_dtypes.bfloat16
    return np.float32


def _build(token):
    dt_in, _ = _DT[token]
    nc = bacc.Bacc("TRN2", target_bir_lowering=False)
    xt = nc.dram_tensor("xt", [KC, 128, RPC], dt_in, kind="ExternalInput")
    mm = nc.dram_tensor("mm", [KC, 128, D], dt_in, kind="ExternalInput")
    y = nc.dram_tensor("y", [RPC, D], mybir.dt.float32, kind="ExternalOutput")

    with tile.TileContext(nc) as tc:
        with (
            tc.tile_pool(name="xp", bufs=1) as xp,
            tc.tile_pool(name="mp", bufs=1) as mp,
            tc.tile_pool(name="op", bufs=3) as op,
            tc.tile_pool(name="pp", bufs=4, space="PSUM") as pp,
        ):
            m_sb = mp.tile([128, KC, D], dt_in)
            nc.sync.dma_start(out=m_sb[:], in_=mm.rearrange("k p d -> p k d"))
            x_sb = []
            for k in range(KC):
                t = xp.tile([128, RPC], dt_in, tag=f"x{k}")
                nc.sync.dma_start(out=t[:], in_=xt[k])
                x_sb.append(t)

            # row index of y = g*(OUT_GROUP*128) + a*128 + p
            yv = y.rearrange("(g a p) d -> g p a d", a=OUT_GROUP, p=128)
            for g in range(NT // OUT_GROUP):
                ot = op.tile([128, OUT_GROUP, D], mybir.dt.float32)
                for a in range(OUT_GROUP):
                    n0 = (g * OUT_GROUP + a) * 128
                    ps = pp.tile([128, D], mybir.dt.float32)
                    for k in range(KC):
                        nc.tensor.matmul(
                            ps[:],
                            x_sb[k][:, n0 : n0 + 128],
                            m_sb[:, k, :],
                            start=(k == 0),
                            stop=(k == KC - 1),
                        )
                    # split PSUM->SBUF copies across DVE and ACT
                    if a % 2 == 0:
                        nc.vector.tensor_copy(ot[:, a, :], ps[:])
                    else:
                        nc.scalar.copy(ot[:, a, :], ps[:])
                nc.sync.dma_start(out=yv[g], in_=ot[:])
    nc.compile()
    return nc


def _fold_m(W_v, s_p, W_p, beta_p, W_o):
    """Host-side constant folding of the tiny parameter tensors into M."""
    W_v = np.asarray(W_v, dtype=np.float64)
    s_p = np.asarray(s_p, dtype=np.float64)
    W_p = np.asarray(W_p, dtype=np.float64)
    beta_p = np.asarray(beta_p, dtype=np.float64)
    W_o = np.asarray(W_o, dtype=np.float64)
    phi = np.einsum("h,dhc,d->hc", s_p, W_p, beta_p)
    norma = np.linalg.norm(phi, axis=1)  # [h]
    M = np.einsum("dhc,h,hce->de", W_v, norma, W_o)  # [512, 512]
    return M.astype(np.float32)


_prog_cache = {}
_last_in_maps = None  # kept for test.py profiling reuse
_last_result = None


def _run(in_maps, token, **kwargs):
    if token not in _prog_cache:
        _prog_cache[token] = _build(token)
    return run_bass_kernel_spmd(_prog_cache[token], in_maps, list(range(N_CORES)), **kwargs)


def kernel(x, W_v, s_p, c_p, W_p, W_A, W_o, beta_p, beta_i_p, **_unused):
    global _last_in_maps, _last_result
    token = COMPUTE_DTYPE
    np_dt = _np_dtype(token)

    x = np.asarray(x, dtype=np.float32)
    M = _fold_m(W_v, s_p, W_p, beta_p, W_o)

    B, N, Dd = x.shape
    assert B * N == ROWS and Dd == D, (x.shape,)

    mmc = np.ascontiguousarray(M.reshape(KC, 128, D)).astype(np_dt)
    xf = x.reshape(ROWS, D)

    in_maps = []
    for c in range(N_CORES):
        sh = xf[c * RPC : (c + 1) * RPC]              # [4096, 512]
        xT = np.ascontiguousarray(sh.T).astype(np_dt)  # [512, 4096]
        in_maps.append({"xt": xT.reshape(KC, 128, RPC), "mm": mmc})

    _last_in_maps = in_maps
    res = _run(in_maps, token)
    _last_result = res
    out = np.concatenate([r["y"] for r in res.results], axis=0)
    return out.reshape(B, N, D)


if __name__ == "__main__":
    # smoke test with random data
    rng = np.random.default_rng(0)
    x = rng.standard_normal((8, 4096, 512)).astype(np.float32)
    W_v = rng.standard_normal((512, 8, 64)).astype(np.float32) * 0.01
    s_p = np.ones((8,), np.float32)
    c_p = np.ones((8,), np.float32)
    W_p = rng.standard_normal((512, 8, 64)).astype(np.float32) * 0.01
    W_A = rng.standard_normal((256, 64)).astype(np.float32)
    W_o = rng.standard_normal((8, 64, 512)).astype(np.float32) * 0.01
    beta_p = rng.standard_normal((512,)).astype(np.float32) * 1e-5
    beta_i_p = rng.standard_normal((4096, 512)).astype(np.float32) * 1e-5
    out = kernel(x, W_v=W_v, s_p=s_p, c_p=c_p, W_p=W_p, W_A=W_A, W_o=W_o,
                 beta_p=beta_p, beta_i_p=beta_i_p)
    M = _fold_m(W_v, s_p, W_p, beta_p, W_o)
    exp = (x.reshape(-1, 512).astype(np.float64) @ M.astype(np.float64)).reshape(8, 4096, 512)
    err = np.abs(out - exp).max() / (np.abs(exp).max() + 1e-30)
    print("smoke rel err:", err)


# revision 7
# speedup vs baseline: 2.1365x; 2.1365x over previous
"""Trainium2 Bass kernel for nn_EstraNet_1443109012284.

Mathematical reduction: the reference's FAVOR+/trig branch (phi_q, aux_q/k,
fr_q/k, aux_A, A) does not feed the output.  The output is exactly

    out[b,n,d] = sum_{h,c} W_o[h,c,d] * norma[h] * sum_{d'} W_v[d',h,c] * x[b,n,d']
               = (x @ M)[b,n,d],   M[d',d] = sum_{h,c} W_v[d',h,c] norma[h] W_o[h,c,d]

with norma[h] = || sum_d s_p[h] W_p[d,h,:] beta_p[d] ||_2.

M is a tiny [512,512] matrix folded on the host; the device does the single
big GEMM  y[32768,512] = x[32768,512] @ M[512,512]  data-parallel over rows:
each of the 8 cores handles 4096 rows.

Device layout per core: x is fed pre-transposed (k-major) so the contraction
dim lands on SBUF partitions without any on-device transpose.  lhsT = xT tile
[128k x 128n] (stationary), rhs = M chunk [128k x 512d] (moving), accumulate
4 k-chunks in PSUM, copy PSUM->SBUF, DMA out in 1 MB blocks.
"""

import sys

sys.path.insert(0, "/opt/trn_rl_repo")

import numpy as np

import concourse.bass as bass
import concourse.tile as tile
from concourse import bacc, mybir
from concourse.bass_utils import run_bass_kernel_spmd

N_CORES = 8
ROWS = 32768          # B*N = 8*4096
RPC = ROWS // N_CORES  # rows per core = 4096
D = 512
KC = 4                # contraction chunks of 128
NT = RPC // 128       # 32 n-tiles of 128 rows per core
OUT_GROUP = 4         # n-tiles per output DMA (1 MB per dma_start)

# device compute dtype: "fp32" (exact, 4 cyc/row), "bf16" (1 cyc/row),
# "f32r" (fp32 data, reduced-precision fast path)
import os as _os
COMPUTE_DTYPE = _os.environ.get("KERNEL_DTYPE", "fp32")

_DT = {
    "fp32": (mybir.dt.float32, np.float32),
    "f32r": (mybir.dt.float32r, np.float32),
    "bf16": (mybir.dt.bfloat16, None),  # numpy dtype resolved via ml_dtypes
}


def _np_dtype(token):
    if token == "bf16":
        import ml_dtypes

        return ml_dtypes.bfloat16
    return np.float32


def _build(token):
    dt_in, _ = _DT[token]
    nc = bacc.Bacc("TRN2", target_bir_lowering=False)
    xt = nc.dram_tensor("xt", [KC, 128, RPC], dt_in, kind="ExternalInput")
    mm = nc.dram_tensor("mm", [KC, 128, D], dt_in, kind="ExternalInput")
    y = nc.dram_tensor("y", [RPC, D], mybir.dt.float32, kind="ExternalOutput")

    with tile.TileContext(nc) as tc:
        with (
            tc.tile_pool(name="xp", bufs=1) as xp,
            tc.tile_pool(name="mp", bufs=1) as mp,
            tc.tile_pool(name="op", bufs=3) as op,
            tc.tile_pool(name="pp", bufs=4, space="PSUM") as pp,
        ):
            m_sb = mp.tile([128, KC, D], dt_in)
            nc.sync.dma_start(out=m_sb[:], in_=mm.rearrange("k p d -> p k d"))
            x_sb = []
            for k in range(KC):
                t = xp.tile([128, RPC], dt_in, tag=f"x{k}")
                nc.sync.dma_start(out=t[:], in_=xt[k])
                x_sb.append(t)

            # row index of y = g*(OUT_GROUP*128) + a*128 + p
            yv = y.rearrange("(g a p) d -> g p a d", a=OUT_GROUP, p=128)
            for g in range(NT // OUT_GROUP):
                ot = op.tile([128, OUT_GROUP, D], mybir.dt.float32)
                for a in range(OUT_GROUP):
                    n0 = (g * OUT_GROUP + a) * 128
                    ps = pp.tile([128, D], mybir.dt.float32)
                    for k in range(KC):
                        nc.tensor.matmul(
                            ps[:],
                            x_sb[k][:, n0 : n0 + 128],
                            m_sb[:, k, :],
                            start=(k == 0),
                            stop=(k == KC - 1),
                        )
                    # split PSUM->SBUF copies across DVE and ACT
                    if a % 2 == 0:
                        nc.vector.tensor_copy(ot[:, a, :], ps[:])
                    else:
                        nc.scalar.copy(ot[:, a, :], ps[:])
                nc.sync.dma_start(out=yv[g], in_=ot[:])
    nc.compile()
    return nc


def _fold_m(W_v, s_p, W_p, beta_p, W_o):
    """Host-side constant folding of the tiny parameter tensors into M."""
    W_v = np.asarray(W_v, dtype=np.float64)
    s_p = np.asarray(s_p, dtype=np.float64)
    W_p = np.asarray(W_p, dtype=np.float64)
    beta_p = np.asarray(beta_p, dtype=np.float64)
    W_o = np.asarray(W_o, dtype=np.float64)
    phi = np.einsum("h,dhc,d->hc", s_p, W_p, beta_p)
    norma = np.linalg.norm(phi, axis=1)  # [h]
    M = np.einsum("dhc,h,hce->de", W_v, norma, W_o)  # [512, 512]
    return M.astype(np.float32)


_prog_cache = {}
_last_in_maps = None  # kept for test.py profiling reuse
_last_result = None


def _run(in_maps, token, **kwargs):
    if token not in _prog_cache:
        _prog_cache[token] = _build(token)
    return run_bass_kernel_spmd(_prog_cache[token], in_maps, list(range(N_CORES)), **kwargs)


def kernel(x, W_v, s_p, c_p, W_p, W_A, W_o, beta_p, beta_i_p, **_unused):
    global _last_in_maps, _last_result
    token = COMPUTE_DTYPE
    np_dt = _np_dtype(token)

    x = np.asarray(x, dtype=np.float32)
    M = _fold_m(W_v, s_p, W_p, beta_p, W_o)

    B, N, Dd = x.shape
    assert B * N == ROWS and Dd == D, (x.shape,)

    mmc = np.ascontiguousarray(M.reshape(KC, 128, D)).astype(np_dt)
    xf = x.reshape(ROWS, D)

    in_maps = []
    for c in range(N_CORES):
        sh = xf[c * RPC : (c + 1) * RPC]              # [4096, 512]
        xT = np.ascontiguousarray(sh.T).astype(np_dt)  # [512, 4096]
        in_maps.append({"xt": xT.reshape(KC, 128, RPC), "mm": mmc})

    _last_in_maps = in_maps
    res = _run(in_maps, token)
    _last_result = res
    out = np.concatenate([r["y"] for r in res.results], axis=0)
    return out.reshape(B, N, D)


if __name__ == "__main__":
    # smoke test with random data
    rng = np.random.default_rng(0)
    x = rng.standard_normal((8, 4096, 512)).astype(np.float32)
    W_v = rng.standard_normal((512, 8, 64)).astype(np.float32) * 0.01
    s_p = np.ones((8,), np.float32)
    c_p = np.ones((8,), np.float32)
    W_p = rng.standard_normal((512, 8, 64)).astype(np.float32) * 0.01
    W_A = rng.standard_normal((256, 64)).astype(np.float32)
    W_o = rng.standard_normal((8, 64, 512)).astype(np.float32) * 0.01
    beta_p = rng.standard_normal((512,)).astype(np.float32) * 1e-5
    beta_i_p = rng.standard_normal((4096, 512)).astype(np.float32) * 1e-5
    out = kernel(x, W_v=W_v, s_p=s_p, c_p=c_p, W_p=W_p, W_A=W_A, W_o=W_o,
                 beta_p=beta_p, beta_i_p=beta_i_p)
    M = _fold_m(W_v, s_p, W_p, beta_p, W_o)
    exp = (x.reshape(-1, 512).astype(np.float64) @ M.astype(np.float64)).reshape(8, 4096, 512)
    err = np.abs(out - exp).max() / (np.abs(exp).max() + 1e-30)
    print("smoke rel err:", err)


# revision 9
# speedup vs baseline: 2.3740x; 1.1112x over previous
"""Trainium2 Bass kernel for nn_EstraNet_1443109012284.

Mathematical reduction: the reference's FAVOR+/trig branch (phi_q, aux_q/k,
fr_q/k, aux_A, A) does not feed the output.  The output is exactly

    out[b,n,d] = sum_{h,c} W_o[h,c,d] * norma[h] * sum_{d'} W_v[d',h,c] * x[b,n,d']
               = (x @ M)[b,n,d],   M[d',d] = sum_{h,c} W_v[d',h,c] norma[h] W_o[h,c,d]

with norma[h] = || sum_d s_p[h] W_p[d,h,:] beta_p[d] ||_2.

M is a tiny [512,512] matrix folded on the host; the device does the single
big GEMM  y[32768,512] = x[32768,512] @ M[512,512]  data-parallel over rows:
each of the 8 cores handles 4096 rows.

Device layout per core: x is fed pre-transposed (k-major) so the contraction
dim lands on SBUF partitions without any on-device transpose.  lhsT = xT tile
[128k x 128n] (stationary), rhs = M chunk [128k x 512d] (moving), accumulate
4 k-chunks in PSUM, copy PSUM->SBUF, DMA out in 1 MB blocks.
"""

import sys

sys.path.insert(0, "/opt/trn_rl_repo")

import numpy as np

import concourse.bass as bass
import concourse.tile as tile
from concourse import bacc, mybir
from concourse.bass_utils import run_bass_kernel_spmd

N_CORES = 8
ROWS = 32768          # B*N = 8*4096
RPC = ROWS // N_CORES  # rows per core = 4096
D = 512
KC = 4                # contraction chunks of 128
NT = RPC // 128       # 32 n-tiles of 128 rows per core
OUT_GROUP = 4         # n-tiles per output DMA (1 MB per dma_start)

# device compute dtype: "fp32" (exact, 4 cyc/row), "bf16" (1 cyc/row),
# "f32r" (fp32 data, reduced-precision fast path)
import os as _os
COMPUTE_DTYPE = _os.environ.get("KERNEL_DTYPE", "fp32")

_DT = {
    "fp32": (mybir.dt.float32, np.float32),
    "f32r": (mybir.dt.float32r, np.float32),
    "bf16": (mybir.dt.bfloat16, None),  # numpy dtype resolved via ml_dtypes
}


def _np_dtype(token):
    if token == "bf16":
        import ml_dtypes

        return ml_dtypes.bfloat16
    return np.float32


N_SLABS = 8
SLAB = RPC // N_SLABS        # 512 rows per slab
TPS = SLAB // 128            # 4 n-tiles of 128 rows per slab


def _build(token):
    dt_in, _ = _DT[token]
    nc = bacc.Bacc("TRN2", target_bir_lowering=False)
    # host feeds x pre-transposed, slab-major: [slab, k-chunk, 128, slab-cols]
    xt = nc.dram_tensor("xt", [N_SLABS, KC, 128, SLAB], dt_in, kind="ExternalInput")
    mm = nc.dram_tensor("mm", [KC, 128, D], dt_in, kind="ExternalInput")
    y = nc.dram_tensor("y", [RPC, D], mybir.dt.float32, kind="ExternalOutput")

    with tile.TileContext(nc) as tc:
        with (
            tc.tile_pool(name="xp", bufs=3) as xp,
            tc.tile_pool(name="mp", bufs=1) as mp,
            tc.tile_pool(name="op", bufs=3) as op,
            tc.tile_pool(name="pp", bufs=4, space="PSUM") as pp,
        ):
            m_sb = mp.tile([128, KC, D], dt_in)
            nc.sync.dma_start(out=m_sb[:], in_=mm.rearrange("k p d -> p k d"))

            # row index of y = s*SLAB + a*128 + p
            yv = y.rearrange("(s a p) d -> s p a d", a=TPS, p=128)
            for s in range(N_SLABS):
                xs = xp.tile([128, KC, SLAB], dt_in)
                nc.sync.dma_start(out=xs[:], in_=xt[s].rearrange("k p w -> p k w"))
                ot = op.tile([128, TPS, D], mybir.dt.float32)
                for a in range(TPS):
                    n0 = a * 128
                    ps = pp.tile([128, D], mybir.dt.float32)
                    for k in range(KC):
                        nc.tensor.matmul(
                            ps[:],
                            xs[:, k, n0 : n0 + 128],
                            m_sb[:, k, :],
                            start=(k == 0),
                            stop=(k == KC - 1),
                        )
                    # split PSUM->SBUF copies across DVE and ACT
                    if a % 2 == 0:
                        nc.vector.tensor_copy(ot[:, a, :], ps[:])
                    else:
                        nc.scalar.copy(ot[:, a, :], ps[:])
                nc.sync.dma_start(out=yv[s], in_=ot[:])
    nc.compile()
    return nc


def _fold_m(W_v, s_p, W_p, beta_p, W_o):
    """Host-side constant folding of the tiny parameter tensors into M."""
    W_v = np.asarray(W_v, dtype=np.float64)
    s_p = np.asarray(s_p, dtype=np.float64)
    W_p = np.asarray(W_p, dtype=np.float64)
    beta_p = np.asarray(beta_p, dtype=np.float64)
    W_o = np.asarray(W_o, dtype=np.float64)
    phi = np.einsum("h,dhc,d->hc", s_p, W_p, beta_p)
    norma = np.linalg.norm(phi, axis=1)  # [h]
    M = np.einsum("dhc,h,hce->de", W_v, norma, W_o)  # [512, 512]
    return M.astype(np.float32)


_prog_cache = {}
_last_in_maps = None  # kept for test.py profiling reuse
_last_result = None


def _run(in_maps, token, **kwargs):
    if token not in _prog_cache:
        _prog_cache[token] = _build(token)
    return run_bass_kernel_spmd(_prog_cache[token], in_maps, list(range(N_CORES)), **kwargs)


def kernel(x, W_v, s_p, c_p, W_p, W_A, W_o, beta_p, beta_i_p, **_unused):
    global _last_in_maps, _last_result
    token = COMPUTE_DTYPE
    np_dt = _np_dtype(token)

    x = np.asarray(x, dtype=np.float32)
    M = _fold_m(W_v, s_p, W_p, beta_p, W_o)

    B, N, Dd = x.shape
    assert B * N == ROWS and Dd == D, (x.shape,)

    mmc = np.ascontiguousarray(M.reshape(KC, 128, D)).astype(np_dt)
    xf = x.reshape(ROWS, D)

    in_maps = []
    for c in range(N_CORES):
        sh = xf[c * RPC : (c + 1) * RPC]               # [4096, 512]
        xT = sh.T.astype(np_dt)                        # [512, 4096] view-transpose
        # [KC, 128, N_SLABS, SLAB] -> [N_SLABS, KC, 128, SLAB]
        xs = np.ascontiguousarray(
            xT.reshape(KC, 128, N_SLABS, SLAB).transpose(2, 0, 1, 3)
        )
        in_maps.append({"xt": xs, "mm": mmc})

    _last_in_maps = in_maps
    res = _run(in_maps, token)
    _last_result = res
    out = np.concatenate([r["y"] for r in res.results], axis=0)
    return out.reshape(B, N, D)


if __name__ == "__main__":
    # smoke test with random data
    rng = np.random.default_rng(0)
    x = rng.standard_normal((8, 4096, 512)).astype(np.float32)
    W_v = rng.standard_normal((512, 8, 64)).astype(np.float32) * 0.01
    s_p = np.ones((8,), np.float32)
    c_p = np.ones((8,), np.float32)
    W_p = rng.standard_normal((512, 8, 64)).astype(np.float32) * 0.01
    W_A = rng.standard_normal((256, 64)).astype(np.float32)
    W_o = rng.standard_normal((8, 64, 512)).astype(np.float32) * 0.01
    beta_p = rng.standard_normal((512,)).astype(np.float32) * 1e-5
    beta_i_p = rng.standard_normal((4096, 512)).astype(np.float32) * 1e-5
    out = kernel(x, W_v=W_v, s_p=s_p, c_p=c_p, W_p=W_p, W_A=W_A, W_o=W_o,
                 beta_p=beta_p, beta_i_p=beta_i_p)
    M = _fold_m(W_v, s_p, W_p, beta_p, W_o)
    exp = (x.reshape(-1, 512).astype(np.float64) @ M.astype(np.float64)).reshape(8, 4096, 512)
    err = np.abs(out - exp).max() / (np.abs(exp).max() + 1e-30)
    print("smoke rel err:", err)


# revision 11
# speedup vs baseline: 2.5201x; 1.0615x over previous
"""Trainium2 Bass kernel for nn_EstraNet_1443109012284.

Mathematical reduction: the reference's FAVOR+/trig branch (phi_q, aux_q/k,
fr_q/k, aux_A, A) does not feed the output.  The output is exactly

    out[b,n,d] = sum_{h,c} W_o[h,c,d] * norma[h] * sum_{d'} W_v[d',h,c] * x[b,n,d']
               = (x @ M)[b,n,d],   M[d',d] = sum_{h,c} W_v[d',h,c] norma[h] W_o[h,c,d]

with norma[h] = || sum_d s_p[h] W_p[d,h,:] beta_p[d] ||_2.

M is a tiny [512,512] matrix folded on the host; the device does the single
big GEMM  y[32768,512] = x[32768,512] @ M[512,512]  data-parallel over rows:
each of the 8 cores handles 4096 rows.

Device design (per core): compute yT = M.T-contracted x, i.e.
    yT[d, n] = sum_k M[k, d] * xT[k, n]
- lhsT (stationary) = M chunk [128k x 128d]  -> only 16 weight loads total,
  each reused for 8 back-to-back matmuls (same-weight MMs pipeline at
  N/2.4GHz; different-weight MMs pay a full array drain each).
- rhs (moving) = xT stripe [128k x 512n], fed pre-transposed from the host
  so no on-device transpose is needed.
- PSUM holds one full d-row-block sweep: 8 banks of [128, 512].
- Output is written as yT [512, 4096] contiguously; host transposes back.
"""

import os as _os
import sys

sys.path.insert(0, "/opt/trn_rl_repo")

import numpy as np

import concourse.bass as bass
import concourse.tile as tile
from concourse import bacc, mybir
from concourse.bass_utils import run_bass_kernel_spmd

N_CORES = 8
ROWS = 32768          # B*N = 8*4096
RPC = ROWS // N_CORES  # rows per core = 4096
D = 512
KC = 4                # contraction chunks of 128
NJ = RPC // 512       # moving chunks of 512 per sweep = 8
DT = D // 128         # output row-blocks = 4

# device compute dtype: "fp32" (exact, 4 cyc/row), "bf16" (1 cyc/row),
# "f32r" (fp32 data, reduced-precision fast path)
COMPUTE_DTYPE = _os.environ.get("KERNEL_DTYPE", "bf16")

_DT = {
    "fp32": mybir.dt.float32,
    "f32r": mybir.dt.float32r,
    "bf16": mybir.dt.bfloat16,
}


def _np_dtype(token):
    if token == "bf16":
        import ml_dtypes

        return ml_dtypes.bfloat16
    return np.float32


def _build(token):
    dt_in = _DT[token]
    nc = bacc.Bacc("TRN2", target_bir_lowering=False)
    xt = nc.dram_tensor("xt", [KC, 128, RPC], dt_in, kind="ExternalInput")
    mm = nc.dram_tensor("mm", [KC, 128, D], dt_in, kind="ExternalInput")
    yt = nc.dram_tensor("yt", [D, RPC], mybir.dt.float32, kind="ExternalOutput")

    with tile.TileContext(nc) as tc:
        with (
            tc.tile_pool(name="xp", bufs=1) as xp,
            tc.tile_pool(name="mp", bufs=1) as mp,
            tc.tile_pool(name="op", bufs=2) as op,
            tc.tile_pool(name="pp", bufs=8, space="PSUM") as pp,
        ):
            m_sb = mp.tile([128, KC, D], dt_in)
            nc.sync.dma_start(out=m_sb[:], in_=mm.rearrange("k p d -> p k d"))
            x_sb = []
            for k in range(KC):
                t = xp.tile([128, RPC], dt_in, tag=f"x{k}")
                nc.sync.dma_start(out=t[:], in_=xt[k])
                x_sb.append(t)

            for d in range(DT):
                d0 = d * 128
                pss = [
                    pp.tile([128, 512], mybir.dt.float32, tag="ps", name=f"ps_{d}_{j}")
                    for j in range(NJ)
                ]
                for k in range(KC):
                    for j in range(NJ):
                        nc.tensor.matmul(
                            pss[j][:],
                            m_sb[:, k, d0 : d0 + 128],
                            x_sb[k][:, j * 512 : (j + 1) * 512],
                            start=(k == 0),
                            stop=(k == KC - 1),
                        )
                ot = op.tile([128, RPC], mybir.dt.float32)
                for j in range(NJ):
                    # split PSUM->SBUF copies across DVE and ACT
                    if j % 2 == 0:
                        nc.vector.tensor_copy(ot[:, j * 512 : (j + 1) * 512], pss[j][:])
                    else:
                        nc.scalar.copy(ot[:, j * 512 : (j + 1) * 512], pss[j][:])
                nc.sync.dma_start(out=yt[d0 : d0 + 128, :], in_=ot[:])
    nc.compile()
    return nc


def _fold_m(W_v, s_p, W_p, beta_p, W_o):
    """Host-side constant folding of the tiny parameter tensors into M."""
    W_v = np.asarray(W_v, dtype=np.float64)
    s_p = np.asarray(s_p, dtype=np.float64)
    W_p = np.asarray(W_p, dtype=np.float64)
    beta_p = np.asarray(beta_p, dtype=np.float64)
    W_o = np.asarray(W_o, dtype=np.float64)
    phi = np.einsum("h,dhc,d->hc", s_p, W_p, beta_p)
    norma = np.linalg.norm(phi, axis=1)  # [h]
    M = np.einsum("dhc,h,hce->de", W_v, norma, W_o)  # [512, 512]
    return M.astype(np.float32)


_prog_cache = {}
_last_in_maps = None  # kept for test.py profiling reuse
_last_result = None


def _run(in_maps, token, **kwargs):
    if token not in _prog_cache:
        _prog_cache[token] = _build(token)
    return run_bass_kernel_spmd(_prog_cache[token], in_maps, list(range(N_CORES)), **kwargs)


def kernel(x, W_v, s_p, c_p, W_p, W_A, W_o, beta_p, beta_i_p, **_unused):
    global _last_in_maps, _last_result
    token = COMPUTE_DTYPE
    np_dt = _np_dtype(token)

    x = np.asarray(x, dtype=np.float32)
    M = _fold_m(W_v, s_p, W_p, beta_p, W_o)

    B, N, Dd = x.shape
    assert B * N == ROWS and Dd == D, (x.shape,)

    mmc = np.ascontiguousarray(M.reshape(KC, 128, D)).astype(np_dt)
    xf = x.reshape(ROWS, D)

    in_maps = []
    for c in range(N_CORES):
        sh = xf[c * RPC : (c + 1) * RPC]               # [4096, 512]
        xT = np.ascontiguousarray(sh.T).astype(np_dt)  # [512, 4096]
        in_maps.append({"xt": xT.reshape(KC, 128, RPC), "mm": mmc})

    _last_in_maps = in_maps
    res = _run(in_maps, token)
    _last_result = res
    out = np.empty((ROWS, D), dtype=np.float32)
    for c in range(N_CORES):
        out[c * RPC : (c + 1) * RPC] = res.results[c]["yt"].T
    return out.reshape(B, N, D)


if __name__ == "__main__":
    # smoke test with random data
    rng = np.random.default_rng(0)
    x = rng.standard_normal((8, 4096, 512)).astype(np.float32)
    W_v = rng.standard_normal((512, 8, 64)).astype(np.float32) * 0.01
    s_p = np.ones((8,), np.float32)
    c_p = np.ones((8,), np.float32)
    W_p = rng.standard_normal((512, 8, 64)).astype(np.float32) * 0.01
    W_A = rng.standard_normal((256, 64)).astype(np.float32)
    W_o = rng.standard_normal((8, 64, 512)).astype(np.float32) * 0.01
    beta_p = rng.standard_normal((512,)).astype(np.float32) * 1e-5
    beta_i_p = rng.standard_normal((4096, 512)).astype(np.float32) * 1e-5
    out = kernel(x, W_v=W_v, s_p=s_p, c_p=c_p, W_p=W_p, W_A=W_A, W_o=W_o,
                 beta_p=beta_p, beta_i_p=beta_i_p)
    M = _fold_m(W_v, s_p, W_p, beta_p, W_o)
    exp = (x.reshape(-1, 512).astype(np.float64) @ M.astype(np.float64)).reshape(8, 4096, 512)
    err = np.abs(out - exp).max() / (np.abs(exp).max() + 1e-30)
    print("smoke rel err:", err)


# revision 13
# speedup vs baseline: 2.8105x; 1.1152x over previous
"""Trainium2 Bass kernel for nn_EstraNet_1443109012284.

Mathematical reduction: the reference's FAVOR+/trig branch (phi_q, aux_q/k,
fr_q/k, aux_A, A) does not feed the output.  The output is exactly

    out[b,n,d] = sum_{h,c} W_o[h,c,d] * norma[h] * sum_{d'} W_v[d',h,c] * x[b,n,d']
               = (x @ M)[b,n,d],   M[d',d] = sum_{h,c} W_v[d',h,c] norma[h] W_o[h,c,d]

with norma[h] = || sum_d s_p[h] W_p[d,h,:] beta_p[d] ||_2.

M is a tiny [512,512] matrix folded on the host; the device does the single
big GEMM  y[32768,512] = x[32768,512] @ M[512,512]  data-parallel over rows:
each of the 8 cores handles 4096 rows.

Device design (per core): compute yT = M.T-contracted x, i.e.
    yT[d, n] = sum_k M[k, d] * xT[k, n]
- lhsT (stationary) = M chunk [128k x 128d]  -> only 16 weight loads total,
  each reused for 8 back-to-back matmuls (same-weight MMs pipeline at
  N/2.4GHz; different-weight MMs pay a full array drain each).
- rhs (moving) = xT stripe [128k x 512n], fed pre-transposed from the host
  so no on-device transpose is needed.
- PSUM holds one full d-row-block sweep: 8 banks of [128, 512].
- Output is written as yT [512, 4096] contiguously; host transposes back.
"""

import os as _os
import sys

sys.path.insert(0, "/opt/trn_rl_repo")

import numpy as np

import concourse.bass as bass
import concourse.tile as tile
from concourse import bacc, mybir
from concourse.bass_utils import run_bass_kernel_spmd

N_CORES = 8
ROWS = 32768          # B*N = 8*4096
RPC = ROWS // N_CORES  # rows per core = 4096
D = 512
KC = 4                # contraction chunks of 128
NJ = RPC // 512       # moving chunks of 512 per sweep = 8
DT = D // 128         # output row-blocks = 4

# device compute dtype: "fp32" (exact, 4 cyc/row), "bf16" (1 cyc/row),
# "f32r" (fp32 data, reduced-precision fast path)
COMPUTE_DTYPE = _os.environ.get("KERNEL_DTYPE", "bf16")

_DT = {
    "fp32": mybir.dt.float32,
    "f32r": mybir.dt.float32r,
    "bf16": mybir.dt.bfloat16,
}


def _np_dtype(token):
    if token == "bf16":
        import ml_dtypes

        return ml_dtypes.bfloat16
    return np.float32


HB = 2                 # n-halves per stripe
HW = RPC // HB         # 2048 columns per half
JH = HW // 512         # 4 moving chunks per phase


def _build(token):
    dt_in = _DT[token]
    nc = bacc.Bacc("TRN2", target_bir_lowering=False)
    # x pre-transposed, [k-chunk, half, 128, 2048] so each half-stripe is one
    # contiguous DMA
    xt = nc.dram_tensor("xt", [KC, HB, 128, HW], dt_in, kind="ExternalInput")
    mm = nc.dram_tensor("mm", [KC, 128, D], dt_in, kind="ExternalInput")
    yt = nc.dram_tensor("yt", [D, RPC], mybir.dt.float32, kind="ExternalOutput")

    with tile.TileContext(nc) as tc:
        with (
            tc.tile_pool(name="xp", bufs=1) as xp,
            tc.tile_pool(name="mp", bufs=1) as mp,
            tc.tile_pool(name="op", bufs=3) as op,
            tc.tile_pool(name="pp", bufs=8, space="PSUM") as pp,
        ):
            m_sb = mp.tile([128, KC, D], dt_in)
            nc.sync.dma_start(out=m_sb[:], in_=mm.rearrange("k p d -> p k d"))
            # load half-stripes in the order compute consumes them
            x_sb = {}
            for h in range(HB):
                for k in range(KC):
                    t = xp.tile([128, HW], dt_in, tag=f"x{k}{h}", name=f"x{k}{h}")
                    nc.sync.dma_start(out=t[:], in_=xt[k, h])
                    x_sb[(k, h)] = t

            # phases: h outer (so the first phase only needs the first 4
            # half-stripe DMAs), d inner; 4 PSUM banks per phase, two phases
            # in flight (bufs=8); copies all on ACT so PE drain + one reader
            # share PSUM without throttling
            for h in range(HB):
                for d in range(DT):
                    d0 = d * 128
                    pss = [
                        pp.tile([128, 512], mybir.dt.float32, tag="ps", name=f"ps_{h}_{d}_{j}")
                        for j in range(JH)
                    ]
                    for k in range(KC):
                        for j in range(JH):
                            nc.tensor.matmul(
                                pss[j][:],
                                m_sb[:, k, d0 : d0 + 128],
                                x_sb[(k, h)][:, j * 512 : (j + 1) * 512],
                                start=(k == 0),
                                stop=(k == KC - 1),
                            )
                    ot = op.tile([128, HW], mybir.dt.float32, name="ot")
                    for j in range(JH):
                        nc.scalar.copy(ot[:, j * 512 : (j + 1) * 512], pss[j][:])
                    nc.sync.dma_start(out=yt[d0 : d0 + 128, h * HW : (h + 1) * HW], in_=ot[:])
    nc.compile()
    return nc


def _fold_m(W_v, s_p, W_p, beta_p, W_o):
    """Host-side constant folding of the tiny parameter tensors into M."""
    W_v = np.asarray(W_v, dtype=np.float64)
    s_p = np.asarray(s_p, dtype=np.float64)
    W_p = np.asarray(W_p, dtype=np.float64)
    beta_p = np.asarray(beta_p, dtype=np.float64)
    W_o = np.asarray(W_o, dtype=np.float64)
    phi = np.einsum("h,dhc,d->hc", s_p, W_p, beta_p)
    norma = np.linalg.norm(phi, axis=1)  # [h]
    M = np.einsum("dhc,h,hce->de", W_v, norma, W_o)  # [512, 512]
    return M.astype(np.float32)


_prog_cache = {}
_last_in_maps = None  # kept for test.py profiling reuse
_last_result = None


def _run(in_maps, token, **kwargs):
    if token not in _prog_cache:
        _prog_cache[token] = _build(token)
    return run_bass_kernel_spmd(_prog_cache[token], in_maps, list(range(N_CORES)), **kwargs)


def kernel(x, W_v, s_p, c_p, W_p, W_A, W_o, beta_p, beta_i_p, **_unused):
    global _last_in_maps, _last_result
    token = COMPUTE_DTYPE
    np_dt = _np_dtype(token)

    x = np.asarray(x, dtype=np.float32)
    M = _fold_m(W_v, s_p, W_p, beta_p, W_o)

    B, N, Dd = x.shape
    assert B * N == ROWS and Dd == D, (x.shape,)

    mmc = np.ascontiguousarray(M.reshape(KC, 128, D)).astype(np_dt)
    xf = x.reshape(ROWS, D)

    in_maps = []
    for c in range(N_CORES):
        sh = xf[c * RPC : (c + 1) * RPC]               # [4096, 512]
        xT = sh.T.astype(np_dt)                        # [512, 4096]
        # [KC, 128, HB, HW] -> [KC, HB, 128, HW], each half-stripe contiguous
        xs = np.ascontiguousarray(
            xT.reshape(KC, 128, HB, HW).transpose(0, 2, 1, 3)
        )
        in_maps.append({"xt": xs, "mm": mmc})

    _last_in_maps = in_maps
    res = _run(in_maps, token)
    _last_result = res
    out = np.empty((ROWS, D), dtype=np.float32)
    for c in range(N_CORES):
        out[c * RPC : (c + 1) * RPC] = res.results[c]["yt"].T
    return out.reshape(B, N, D)


if __name__ == "__main__":
    # smoke test with random data
    rng = np.random.default_rng(0)
    x = rng.standard_normal((8, 4096, 512)).astype(np.float32)
    W_v = rng.standard_normal((512, 8, 64)).astype(np.float32) * 0.01
    s_p = np.ones((8,), np.float32)
    c_p = np.ones((8,), np.float32)
    W_p = rng.standard_normal((512, 8, 64)).astype(np.float32) * 0.01
    W_A = rng.standard_normal((256, 64)).astype(np.float32)
    W_o = rng.standard_normal((8, 64, 512)).astype(np.float32) * 0.01
    beta_p = rng.standard_normal((512,)).astype(np.float32) * 1e-5
    beta_i_p = rng.standard_normal((4096, 512)).astype(np.float32) * 1e-5
    out = kernel(x, W_v=W_v, s_p=s_p, c_p=c_p, W_p=W_p, W_A=W_A, W_o=W_o,
                 beta_p=beta_p, beta_i_p=beta_i_p)
    M = _fold_m(W_v, s_p, W_p, beta_p, W_o)
    exp = (x.reshape(-1, 512).astype(np.float64) @ M.astype(np.float64)).reshape(8, 4096, 512)
    err = np.abs(out - exp).max() / (np.abs(exp).max() + 1e-30)
    print("smoke rel err:", err)
